# revision 39
# baseline (speedup 1.0000x reference)
"""Trainium2 Bass kernel for nn_AdaGMNConv (gnn_message_passing).

Sharding: one graph per NeuronCore (G=8 graphs, 8 cores). All compute is
local to a core; the host gathers the per-graph scalar outputs.

Per-core math (graph g, M=2048 high-degree nodes per graph, D=128):
  A    = H_g @ F^T                      [2048, 2048]   (bf16 matmul, f32 psum)
  A1   = segment softmax of A over rows (per column)   -> S1 = A1 @ F
  A2   = softmax of A over columns (per row)           -> S2 = A2^T @ H
  out_multi  = MLP([H | S1]); out_single = MLP([F | S2])
  p2 = colsum(out_multi) + colsum(L_g);  p1 = colsum(out_single) + colsum(L_gid)
  out[g] = <p1/||p1||, p2/||p2||>

Key structure:
  - ONE exp pass: a single per-core scalar shift c_g (host-computed from the
    gid block's row norms; margins are huge for this data) makes the two
    softmax orientations share E = exp(A - c). E1 [j,i] comes from the matmul
    + fused ScalarE exp (denominator accumulated for free); E2 [i,j] is a DMA
    xbar transpose of E1 (idle DMA engines), with rows in (p t)-interleaved
    order matched by a permuted H load.
  - den2 (column sums of E1) via ones-vector matmuls on TensorE, reshaped to
    per-partition layout by a DMA.
  - Softmax divisions are folded into the small F/H matmul operands.
  - The MLP's second linear layer collapses onto the pooled vector (only
    column sums of the MLP output are ever needed); LayerNorm rstd is a
    batched exp(-0.5*ln(var+eps)) so every ACT op lives in one table set.
"""

import os
from contextlib import ExitStack

import numpy as np

import concourse.bass as bass
import concourse.tile as tile
from concourse.tile import add_dep_helper
from concourse import mybir
from concourse.bass_utils import run_bass_kernel_spmd

f32 = mybir.dt.float32
bf16 = mybir.dt.bfloat16

P = 128          # partitions
D = 128          # feature dim
NT = 16          # tiles per 2048-node block
M = P * NT       # 2048 nodes per block
SHIFT0 = 64.0    # exp shift for non-gid cores
LN_EPS = 1e-5
CH = 1024        # PSUM chunk width for the attention tiles (2 banks)
MMN = 512        # matmul moving free-dim (one PSUM bank)

MAXW = 1  # walrus in this env rejects >1 sem-wait per instruction


def split_waits(nc, maxw=MAXW):
    """Hoist overflow sem-waits onto preceding same-engine NOPs (this walrus
    build only accepts `maxw` waits per instruction)."""
    ctr = 0
    for fn in nc.m.functions:
        for bb in fn.blocks:
            new_insts = []
            for inst in bb.instructions:
                si = inst.sync_info
                if si is not None and si.on_wait and len(si.on_wait) > maxw:
                    waits = list(si.on_wait)
                    chunks = [waits[i : i + maxw] for i in range(0, len(waits), maxw)]
                    for ch in chunks[:-1]:
                        ctr += 1
                        nop = mybir.InstNoOp(
                            name=f"waitsplit_{ctr}",
                            sync_info=mybir.SyncInfo(on_wait=ch, on_update=[]),
                            bass_nofuse=True,
                            engine=inst.engine,
                        )
                        new_insts.append(nop)
                    si.on_wait = chunks[-1]
                new_insts.append(inst)
            bb.instructions = new_insts
    return ctr


def build_nc(has_b1, has_b2, has_gamma, has_beta):
    nc = bass.Bass()

    # ---- DRAM parameters (per-core shard shapes) ----
    dHTb = nc.declare_dram_parameter("HTb", [D, M], bf16, isOutput=False)
    dFTb = nc.declare_dram_parameter("FTb", [D, M], bf16, isOutput=False)
    dH = nc.declare_dram_parameter("H", [M, D], f32, isOutput=False)
    dF = nc.declare_dram_parameter("F", [M, D], f32, isOutput=False)
    dL = nc.declare_dram_parameter("L", [M, D], f32, isOutput=False)
    dL0 = nc.declare_dram_parameter("L0", [M, D], f32, isOutput=False)
    dW1 = nc.declare_dram_parameter("W1b", [2 * D, D], bf16, isOutput=False)
    dW2 = nc.declare_dram_parameter("W2", [D, D], f32, isOutput=False)
    dNEG = nc.declare_dram_parameter("NEG", [P, 1], f32, isOutput=False)
    dB1 = dB2 = dGAM = dBET = None
    if has_b1:
        dB1 = nc.declare_dram_parameter("B1b", [1, D], bf16, isOutput=False)
    if has_b2:
        dB2 = nc.declare_dram_parameter("B2", [1, D], f32, isOutput=False)
    if has_gamma:
        dGAM = nc.declare_dram_parameter("GAM", [1, D], f32, isOutput=False)
    if has_beta:
        dBET = nc.declare_dram_parameter("BET", [1, D], f32, isOutput=False)
    dOUT = nc.declare_dram_parameter("out", [1, 1], f32, isOutput=True)

    with tile.TileContext(nc) as tc, ExitStack() as ctx:
        consts = ctx.enter_context(tc.tile_pool(name="consts", bufs=1))
        scal = ctx.enter_context(tc.tile_pool(name="scal", bufs=4))
        fpp = ctx.enter_context(tc.tile_pool(name="fpp", bufs=3))
        mlpt = ctx.enter_context(tc.tile_pool(name="mlpt", bufs=3))
        # PSUM budget (8 banks): psA = 2 slots x [128,1024] (4 banks) shared by
        # A-chunk tiles, den2 groups, MLP pre-act tiles and tail matmuls;
        # psS = 1 slot x [128,2048] (4 banks) for the S1T/S2T accumulators.
        psA = ctx.enter_context(tc.tile_pool(name="psA", bufs=2, space="PSUM"))
        psS = ctx.enter_context(tc.tile_pool(name="psS", bufs=1, space="PSUM"))

        # ---- SBUF loads: attention operands first, split per moving chunk ----
        sb_FTb = consts.tile([P, M], bf16)
        for c in range(4):
            cs = bass.ts(c, M // 4)
            nc.sync.dma_start(out=sb_FTb[:, cs], in_=dFTb[:, cs])
        # moving operand: one tile per 512-chunk so the first matmuls can
        # start as soon as their own chunk has landed
        sb_HTc = [consts.tile([P, MMN], bf16, name=f"HTc{c}", tag=f"HTc{c}")
                  for c in range(M // MMN)]
        for c, t_ in enumerate(sb_HTc):
            nc.sync.dma_start(out=t_, in_=dHTb[:, bass.ts(c, MMN)])
        sb_NEG = consts.tile([P, 1], f32)
        nc.sync.dma_start(out=sb_NEG, in_=dNEG[:, :])
        sb_F = consts.tile([P, NT, D], f32)
        dFr = dF[:, :].rearrange("(t p) d -> p t d", p=P)
        for c in range(2):
            nc.sync.dma_start(out=sb_F[:, bass.ts(c, NT // 2), :],
                              in_=dFr[:, bass.ts(c, NT // 2), :])
        sb_W1b = consts.tile([P, 2, D], bf16)
        nc.sync.dma_start(out=sb_W1b, in_=dW1[:, :].rearrange("(t p) d -> p t d", p=P))
        # E2's transpose layout keeps natural i-blocks: H loads naturally
        sb_Hp = consts.tile([P, NT, D], f32)
        dHr = dH[:, :].rearrange("(t p) d -> p t d", p=P)
        for c in range(2):
            nc.sync.dma_start(out=sb_Hp[:, bass.ts(c, NT // 2), :],
                              in_=dHr[:, bass.ts(c, NT // 2), :])
        sb_W2 = consts.tile([P, D], f32)
        nc.sync.dma_start(out=sb_W2, in_=dW2[:, :])
        sb_L = consts.tile([P, NT, D], f32)
        nc.sync.dma_start(out=sb_L, in_=dL[:, :].rearrange("(t p) d -> p t d", p=P))
        sb_L0 = consts.tile([P, NT, D], f32)
        nc.sync.dma_start(out=sb_L0, in_=dL0[:, :].rearrange("(t p) d -> p t d", p=P))

        sb_B1b = sb_B2 = None
        if has_b1:
            sb_B1b = consts.tile([1, D], bf16)
            nc.sync.dma_start(out=sb_B1b, in_=dB1[:, :])
        if has_b2:
            sb_B2 = consts.tile([1, D], f32)
            nc.sync.dma_start(out=sb_B2, in_=dB2[:, :])
        gam_bc = bet_bc = None
        if has_gamma:
            gam_bc = consts.tile([P, D], f32)
            src = dGAM[:, :]
            nc.sync.dma_start(
                out=gam_bc,
                in_=bass.AP(tensor=src.tensor, offset=src.offset,
                            ap=[[0, P], src.ap[1]]),
            )
        if has_beta:
            bet_bc = consts.tile([P, D], f32)
            src = dBET[:, :]
            nc.sync.dma_start(
                out=bet_bc,
                in_=bass.AP(tensor=src.tensor, offset=src.offset,
                            ap=[[0, P], src.ap[1]]),
            )

        ones_f = consts.tile([P, 1], f32)
        nc.vector.memset(ones_f, 1.0)
        ones_b = consts.tile([P, 1], bf16)
        nc.vector.memset(ones_b, 1.0)
        sb_eps = consts.tile([P, 1], f32)
        nc.vector.memset(sb_eps, LN_EPS)
        ones_row = consts.tile([1, D], bf16)
        nc.vector.memset(ones_row, 1.0)
        c2048 = consts.tile([1, 1], f32)
        nc.vector.memset(c2048, float(M))

        sb_E1 = consts.tile([P, NT, M], bf16)
        sb_E2 = consts.tile([P, NT, M], bf16)
        sb_S1Tb = consts.tile([P, M], bf16)
        sb_S2Tb = consts.tile([P, M], bf16)

        # ---- Phase 1: A^T tiles -> exp (E1 + den1) -> F' -> S1T; transpose ----
        ps_s1t = psS.tile([P, M], f32, tag="psS")
        for t in range(NT):
            tr = bass.ts(t, P)
            dparts = scal.tile([P, M // CH], f32, tag="dparts")
            for c in range(M // CH):
                pa = psA.tile([P, CH], f32, tag="psA")
                for h in range(CH // MMN):
                    ci = c * (CH // MMN) + h
                    nc.tensor.matmul(
                        pa[:, bass.ts(h, MMN)], lhsT=sb_FTb[:, tr],
                        rhs=sb_HTc[ci], start=True, stop=True,
                    )
                nc.scalar.activation(
                    out=sb_E1[:, t, bass.ts(c, CH)], in_=pa,
                    func=mybir.ActivationFunctionType.Exp,
                    bias=sb_NEG, scale=1.0,
                    accum_out=dparts[:, c : c + 1],
                )
            # transpose E1 tile -> E2 column block (idle DMA engines)
            nc.sync.dma_start_transpose(out=sb_E2[:, :, tr], in_=sb_E1[:, t, :])
            den = scal.tile([P, 1], f32, tag="den")
            nc.vector.reduce_sum(out=den, in_=dparts, axis=mybir.AxisListType.X)
            rec = scal.tile([P, 1], f32, tag="rec")
            nc.vector.reciprocal(out=rec, in_=den)
            fp = fpp.tile([P, D], bf16, tag="fp")
            nc.vector.tensor_scalar_mul(out=fp, in0=sb_F[:, t, :], scalar1=rec)
            for h in range(M // MMN):
                hs = bass.ts(h, MMN)
                nc.tensor.matmul(
                    ps_s1t[:, hs], lhsT=fp, rhs=sb_E1[:, t, hs],
                    start=(t == 0), stop=(t == NT - 1),
                )
        s1t_evac = nc.vector.tensor_copy(out=sb_S1Tb, in_=ps_s1t)

        # ---- den2[i] = column sums of E1 via ones-matmuls; each group's
        # [1, 512] PSUM row is scattered to per-partition layout by a DMA ----
        # Explicitly ordered after phase 1 so the accumulation groups don't
        # grab a psA slot mid-phase and starve the A-chunk pipeline.
        den2p = consts.tile([P, NT], f32)
        den2rec = consts.tile([P, NT], f32)
        sb_d2row = consts.tile([1, M], f32)
        ngrp = MMN // P
        for c in range(M // MMN):
            ps_den = psA.tile([1, MMN], f32, tag="psA")
            for t in range(NT):
                mm = nc.tensor.matmul(
                    ps_den, lhsT=ones_b, rhs=sb_E1[:, t, bass.ts(c, MMN)],
                    start=(t == 0), stop=(t == NT - 1),
                )
                if t == 0:
                    add_dep_helper(mm.ins, s1t_evac.ins, sync=False,
                                   reason="den2 group after phase 1")
            row = sb_d2row[:, bass.ts(c, MMN)]
            nc.vector.tensor_copy(out=row, in_=ps_den)
            for tt in range(ngrp):
                tcol = c * ngrp + tt
                nc.gpsimd.dma_start(
                    out=den2p[:, tcol : tcol + 1],
                    in_=sb_d2row[:, bass.ds(tcol * P, P)],
                )
            nc.vector.reciprocal(out=den2rec[:, bass.ts(c, ngrp)],
                                 in_=den2p[:, bass.ts(c, ngrp)])

        # ---- Phase 2: H' groups @ E2 -> S2T accumulation ----
        ps_s2t = psS.tile([P, M], f32, tag="psS")
        for c in range(NT):
            hp = fpp.tile([P, D], bf16, tag="fp")
            nc.vector.tensor_scalar_mul(out=hp, in0=sb_Hp[:, c, :],
                                        scalar1=den2rec[:, c : c + 1])
            for h in range(M // MMN):
                hs = bass.ts(h, MMN)
                nc.tensor.matmul(
                    ps_s2t[:, hs], lhsT=hp, rhs=sb_E2[:, c, hs],
                    start=(c == 0), stop=(c == NT - 1),
                )
        nc.vector.tensor_copy(out=sb_S2Tb, in_=ps_s2t)

        # ---- MLP phases: pre-act -> LayerNorm -> ReLU -> column-sum ----
        def mlp_colsum(xslice, sTb, pre_all, mvall, racc, r_sb):
            for t in range(NT):
                tr = bass.ts(t, P)
                pre = psA.tile([P, D], f32, tag="psA")
                nc.tensor.matmul(pre, lhsT=xslice(t), rhs=sb_W1b[:, 0, :],
                                 start=True, stop=False)
                nc.tensor.matmul(pre, lhsT=sTb[:, tr], rhs=sb_W1b[:, 1, :],
                                 start=False, stop=not has_b1)
                if has_b1:
                    nc.tensor.matmul(pre, lhsT=ones_row, rhs=sb_B1b,
                                     start=False, stop=True)
                nc.scalar.copy(out=pre_all[:, t, :], in_=pre)
                stats = scal.tile([P, 6], f32, tag="stats")
                nc.vector.bn_stats(out=stats, in_=pre_all[:, t, :])
                nc.vector.bn_aggr(out=mvall[:, t, :], in_=stats)
            lnv = scal.tile([P, NT], f32, tag="lnv")
            nc.scalar.activation(out=lnv, in_=mvall[:, :, 1],
                                 func=mybir.ActivationFunctionType.Ln,
                                 bias=sb_eps, scale=1.0)
            rstd_all = consts.tile([P, NT], f32)
            nc.scalar.activation(out=rstd_all, in_=lnv,
                                 func=mybir.ActivationFunctionType.Exp,
                                 scale=-0.5)
            for t in range(NT):
                tt = mlpt.tile([P, D], f32, tag="tt")
                nc.vector.tensor_scalar(
                    out=tt, in0=pre_all[:, t, :], scalar1=mvall[:, t, 0:1],
                    scalar2=rstd_all[:, t : t + 1],
                    op0=mybir.AluOpType.subtract, op1=mybir.AluOpType.mult,
                )
                if has_gamma:
                    nc.vector.tensor_mul(out=tt, in0=tt, in1=gam_bc)
                if has_beta:
                    nc.vector.tensor_add(out=tt, in0=tt, in1=bet_bc)
                if t == 0:
                    nc.vector.tensor_scalar_max(out=racc, in0=tt, scalar1=0.0)
                else:
                    nc.vector.scalar_tensor_tensor(
                        out=racc, in0=tt, scalar=0.0, in1=racc,
                        op0=mybir.AluOpType.max, op1=mybir.AluOpType.add,
                    )
            ps_r = psA.tile([P, 1], f32, tag="psA")
            nc.tensor.matmul(ps_r, lhsT=racc, rhs=ones_f, start=True, stop=True)
            nc.vector.tensor_copy(out=r_sb, in_=ps_r)

        nchunk = MMN // P

        pre_all2 = consts.tile([P, NT, D], bf16)
        mvall2 = consts.tile([P, NT, 2], f32)
        racc2 = consts.tile([P, D], f32)
        r2_sb = consts.tile([P, 1], f32)
        mlp_colsum(lambda t: sb_HTc[t // nchunk][:, bass.ts(t % nchunk, P)],
                   sb_S1Tb, pre_all2, mvall2, racc2, r2_sb)

        # L column sums emitted here: they fill TensorE bubbles while the
        # MLP LayerNorm chains run on VectorE/ScalarE
        def lcolsum(sb_src, l_sb):
            ps_l = psA.tile([P, 1], f32, tag="psA")
            for t in range(NT):
                nc.tensor.matmul(ps_l, lhsT=sb_src[:, t, :], rhs=ones_f,
                                 start=(t == 0), stop=(t == NT - 1))
            nc.vector.tensor_copy(out=l_sb, in_=ps_l)

        l2_sb = consts.tile([P, 1], f32)
        lcolsum(sb_L, l2_sb)
        l1_sb = consts.tile([P, 1], f32)
        lcolsum(sb_L0, l1_sb)

        pre_all1 = consts.tile([P, NT, D], bf16)
        mvall1 = consts.tile([P, NT, 2], f32)
        racc1 = consts.tile([P, D], f32)
        r1_sb = consts.tile([P, 1], f32)
        mlp_colsum(lambda t: sb_FTb[:, bass.ts(t, P)],
                   sb_S2Tb, pre_all1, mvall1, racc1, r1_sb)

        # ---- pooled vectors ----
        pcat = consts.tile([P, 2], f32)

        def pvec(r_sb, l_sb, out_slice):
            ps_p = psA.tile([P, 1], f32, tag="psA")
            nc.tensor.matmul(ps_p, lhsT=sb_W2, rhs=r_sb, start=True,
                             stop=not has_b2)
            if has_b2:
                nc.tensor.matmul(ps_p, lhsT=sb_B2, rhs=c2048, start=False,
                                 stop=True)
            nc.vector.tensor_add(out=out_slice, in0=ps_p, in1=l_sb)

        pvec(r2_sb, l2_sb, pcat[:, 1:2])
        pvec(r1_sb, l1_sb, pcat[:, 0:1])

        # ---- final: out = s12 / sqrt(s11*s22) via exp(-0.5*ln(.)) ----
        ps_d1 = psA.tile([1, 2], f32, tag="psA")
        nc.tensor.matmul(ps_d1, lhsT=pcat[:, 0:1], rhs=pcat, start=True, stop=True)
        ps_d2 = psA.tile([1, 1], f32, tag="psA")
        nc.tensor.matmul(ps_d2, lhsT=pcat[:, 1:2], rhs=pcat[:, 1:2],
                         start=True, stop=True)
        dots = consts.tile([1, 4], f32)
        nc.vector.tensor_copy(out=dots[:, 0:2], in_=ps_d1)   # s11, s12
        nc.vector.tensor_copy(out=dots[:, 2:3], in_=ps_d2)   # s22
        q = consts.tile([1, 1], f32)
        nc.vector.tensor_mul(out=q, in0=dots[:, 0:1], in1=dots[:, 2:3])
        nc.vector.tensor_scalar_max(out=q, in0=q, scalar1=1e-30)
        lq = consts.tile([1, 1], f32)
        nc.scalar.activation(out=lq, in_=q,
                             func=mybir.ActivationFunctionType.Ln)
        rq = consts.tile([1, 1], f32)
        nc.scalar.activation(out=rq, in_=lq,
                             func=mybir.ActivationFunctionType.Exp,
                             scale=-0.5)
        res = consts.tile([1, 1], f32)
        nc.vector.tensor_mul(out=res, in0=dots[:, 1:2], in1=rq)
        nc.sync.dma_start(out=dOUT[:, :], in_=res)

    split_waits(nc)
    return nc


_BUILD_CACHE = {}


def _get_nc(flags):
    if flags not in _BUILD_CACHE:
        _BUILD_CACHE[flags] = build_nc(*flags)
    return _BUILD_CACHE[flags]


def kernel(x, edge_attr, W1, b1, gamma, beta, W2, b2, gid, edge_index, batch):
    import ml_dtypes

    nbf16 = ml_dtypes.bfloat16
    x = np.asarray(x, dtype=np.float32)
    W1 = np.asarray(W1, dtype=np.float32)
    b1 = np.asarray(b1, dtype=np.float32)
    gamma = np.asarray(gamma, dtype=np.float32)
    beta = np.asarray(beta, dtype=np.float32)
    W2 = np.asarray(W2, dtype=np.float32)
    b2 = np.asarray(b2, dtype=np.float32)
    gid = int(np.asarray(gid))
    ei0 = np.asarray(edge_index)[0]
    b = np.asarray(batch)

    N, Dx = x.shape
    assert Dx == D
    deg = np.bincount(ei0, minlength=N)
    mask = deg > 1
    G = int(b.max()) + 1
    assert G == 8
    hd_idx = np.where(mask)[0]
    fhb = b[hd_idx]
    Mtot = hd_idx.size
    assert Mtot % G == 0 and np.array_equal(
        fhb, np.repeat(np.arange(G), Mtot // G)
    )
    assert Mtot // G == M

    gxf_idx = np.where(mask & (b == gid))[0]
    assert gxf_idx.size == M
    F = np.ascontiguousarray(x[gxf_idx])
    FTb = np.ascontiguousarray(F.T).astype(nbf16)
    lo0_idx = np.where((~mask) & (b == gid))[0]
    assert lo0_idx.size == M
    L0 = np.ascontiguousarray(x[lo0_idx])

    # per-core scalar exp shift: gid core centers the window on the row-norm
    # range (diagonal dominates there); others use a constant
    sq = (F.astype(np.float64) ** 2).sum(1)
    c_gid = float((sq.max() + sq.min()) / 2.0)

    flags = (
        bool(np.any(b1 != 0.0)),
        bool(np.any(b2 != 0.0)),
        bool(np.any(gamma != 1.0)),
        bool(np.any(beta != 0.0)),
    )
    has_b1, has_b2, has_gamma, has_beta = flags
    nc = _get_nc(flags)

    W1b = W1.astype(nbf16)
    in_maps = []
    for g in range(G):
        sel_h = mask & (b == g)
        sel_l = (~mask) & (b == g)
        assert sel_h.sum() == M and sel_l.sum() == M
        H = np.ascontiguousarray(x[sel_h])
        L = np.ascontiguousarray(x[sel_l])
        cshift = c_gid if g == gid else SHIFT0
        im = {
            "HTb": np.ascontiguousarray(H.T).astype(nbf16),
            "FTb": FTb,
            "H": H,
            "F": F,
            "L": L,
            "L0": L0,
            "W1b": W1b,
            "W2": W2,
            "NEG": np.full((P, 1), -cshift, np.float32),
        }
        if has_b1:
            im["B1b"] = b1.reshape(1, D).astype(nbf16)
        if has_b2:
            im["B2"] = b2.reshape(1, D).astype(np.float32)
        if has_gamma:
            im["GAM"] = gamma.reshape(1, D).astype(np.float32)
        if has_beta:
            im["BET"] = beta.reshape(1, D).astype(np.float32)
        in_maps.append(im)

    trace_dir = os.environ.get("ADAGMN_TRACE", "")
    if trace_dir:
        res = run_bass_kernel_spmd(
            nc, in_maps, core_ids=list(range(G)), trace=True, tmpdir=trace_dir
        )
        print(f"HW exec time: {res.exec_time_ns} ns")
    else:
        res = run_bass_kernel_spmd(nc, in_maps, core_ids=list(range(G)))
    out = np.array([res.results[g]["out"][0, 0] for g in range(G)], np.float32)
    return out


# revision 40
# speedup vs baseline: 1.1494x; 1.1494x over previous
"""Trainium2 Bass kernel for nn_AdaGMNConv (gnn_message_passing).

Sharding: one graph per NeuronCore (G=8 graphs, 8 cores). All compute is
local to a core; the host gathers the per-graph scalar outputs.

Per-core math (graph g, M=2048 high-degree nodes per graph, D=128):
  A    = H_g @ F^T                      [2048, 2048]   (bf16 matmul, f32 psum)
  A1   = segment softmax of A over rows (per column)   -> S1 = A1 @ F
  A2   = softmax of A over columns (per row)           -> S2 = A2^T @ H
  out_multi  = MLP([H | S1]); out_single = MLP([F | S2])
  p2 = colsum(out_multi) + colsum(L_g);  p1 = colsum(out_single) + colsum(L_gid)
  out[g] = <p1/||p1||, p2/||p2||>

Key structure:
  - ONE exp pass: a single per-core scalar shift c_g (host-computed from the
    gid block's row norms; margins are huge for this data) makes the two
    softmax orientations share E = exp(A - c). E1 [j,i] comes from the matmul
    + fused ScalarE exp (denominator accumulated for free); E2 [i,j] is a DMA
    xbar transpose of E1 (idle DMA engines), with rows in (p t)-interleaved
    order matched by a permuted H load.
  - den2 (column sums of E1) via ones-vector matmuls on TensorE, reshaped to
    per-partition layout by a DMA.
  - Softmax divisions are folded into the small F/H matmul operands.
  - The MLP's second linear layer collapses onto the pooled vector (only
    column sums of the MLP output are ever needed); LayerNorm rstd is a
    batched exp(-0.5*ln(var+eps)) so every ACT op lives in one table set.
"""

import os
from contextlib import ExitStack

import numpy as np

import concourse.bass as bass
import concourse.tile as tile
from concourse.tile import add_dep_helper
from concourse import mybir
from concourse.bass_utils import run_bass_kernel_spmd

f32 = mybir.dt.float32
bf16 = mybir.dt.bfloat16

P = 128          # partitions
D = 128          # feature dim
NT = 16          # tiles per 2048-node block
M = P * NT       # 2048 nodes per block
SHIFT0 = 64.0    # exp shift for non-gid cores
LN_EPS = 1e-5
CH = 1024        # PSUM chunk width for the attention tiles (2 banks)
MMN = 512        # matmul moving free-dim (one PSUM bank)

MAXW = 1  # walrus in this env rejects >1 sem-wait per instruction


def split_waits(nc, maxw=MAXW):
    """Hoist overflow sem-waits onto preceding same-engine NOPs (this walrus
    build only accepts `maxw` waits per instruction)."""
    ctr = 0
    for fn in nc.m.functions:
        for bb in fn.blocks:
            new_insts = []
            for inst in bb.instructions:
                si = inst.sync_info
                if si is not None and si.on_wait and len(si.on_wait) > maxw:
                    waits = list(si.on_wait)
                    chunks = [waits[i : i + maxw] for i in range(0, len(waits), maxw)]
                    for ch in chunks[:-1]:
                        ctr += 1
                        nop = mybir.InstNoOp(
                            name=f"waitsplit_{ctr}",
                            sync_info=mybir.SyncInfo(on_wait=ch, on_update=[]),
                            bass_nofuse=True,
                            engine=inst.engine,
                        )
                        new_insts.append(nop)
                    si.on_wait = chunks[-1]
                new_insts.append(inst)
            bb.instructions = new_insts
    return ctr


def build_nc(has_b1, has_b2, has_gamma, has_beta):
    nc = bass.Bass()

    # ---- DRAM parameters (per-core shard shapes) ----
    dHTb = nc.declare_dram_parameter("HTb", [D, M], bf16, isOutput=False)
    dFTb = nc.declare_dram_parameter("FTb", [D, M], bf16, isOutput=False)
    dH = nc.declare_dram_parameter("H", [M, D], f32, isOutput=False)
    dF = nc.declare_dram_parameter("F", [M, D], f32, isOutput=False)
    dL = nc.declare_dram_parameter("L", [M, D], f32, isOutput=False)
    dL0 = nc.declare_dram_parameter("L0", [M, D], f32, isOutput=False)
    dW1 = nc.declare_dram_parameter("W1b", [2 * D, D], bf16, isOutput=False)
    dW2 = nc.declare_dram_parameter("W2", [D, D], f32, isOutput=False)
    dNEG = nc.declare_dram_parameter("NEG", [P, 1], f32, isOutput=False)
    dB1 = dB2 = dGAM = dBET = None
    if has_b1:
        dB1 = nc.declare_dram_parameter("B1b", [1, D], bf16, isOutput=False)
    if has_b2:
        dB2 = nc.declare_dram_parameter("B2", [1, D], f32, isOutput=False)
    if has_gamma:
        dGAM = nc.declare_dram_parameter("GAM", [1, D], f32, isOutput=False)
    if has_beta:
        dBET = nc.declare_dram_parameter("BET", [1, D], f32, isOutput=False)
    dOUT = nc.declare_dram_parameter("out", [1, 1], f32, isOutput=True)

    with tile.TileContext(nc) as tc, ExitStack() as ctx:
        consts = ctx.enter_context(tc.tile_pool(name="consts", bufs=1))
        scal = ctx.enter_context(tc.tile_pool(name="scal", bufs=4))
        fpp = ctx.enter_context(tc.tile_pool(name="fpp", bufs=3))
        mlpt = ctx.enter_context(tc.tile_pool(name="mlpt", bufs=3))
        # PSUM budget (8 banks): psA = 2 slots x [128,1024] (4 banks) shared by
        # A-chunk tiles, den2 groups, MLP pre-act tiles and tail matmuls;
        # psS = 1 slot x [128,2048] (4 banks) for the S1T/S2T accumulators.
        psA = ctx.enter_context(tc.tile_pool(name="psA", bufs=2, space="PSUM"))
        psS = ctx.enter_context(tc.tile_pool(name="psS", bufs=1, space="PSUM"))

        # ---- SBUF loads: attention operands first, split per moving chunk ----
        sb_FTb = consts.tile([P, M], bf16)
        for c in range(4):
            cs = bass.ts(c, M // 4)
            nc.sync.dma_start(out=sb_FTb[:, cs], in_=dFTb[:, cs])
        # moving operand: one tile per 512-chunk so the first matmuls can
        # start as soon as their own chunk has landed
        sb_HTc = [consts.tile([P, MMN], bf16, name=f"HTc{c}", tag=f"HTc{c}")
                  for c in range(M // MMN)]
        for c, t_ in enumerate(sb_HTc):
            nc.sync.dma_start(out=t_, in_=dHTb[:, bass.ts(c, MMN)])
        sb_NEG = consts.tile([P, 1], f32)
        nc.sync.dma_start(out=sb_NEG, in_=dNEG[:, :])
        sb_F = consts.tile([P, NT, D], f32)
        dFr = dF[:, :].rearrange("(t p) d -> p t d", p=P)
        for c in range(2):
            nc.sync.dma_start(out=sb_F[:, bass.ts(c, NT // 2), :],
                              in_=dFr[:, bass.ts(c, NT // 2), :])
        sb_W1b = consts.tile([P, 2, D], bf16)
        nc.sync.dma_start(out=sb_W1b, in_=dW1[:, :].rearrange("(t p) d -> p t d", p=P))
        # E2's transpose layout keeps natural i-blocks: H loads naturally
        sb_Hp = consts.tile([P, NT, D], f32)
        dHr = dH[:, :].rearrange("(t p) d -> p t d", p=P)
        for c in range(2):
            nc.sync.dma_start(out=sb_Hp[:, bass.ts(c, NT // 2), :],
                              in_=dHr[:, bass.ts(c, NT // 2), :])
        sb_W2 = consts.tile([P, D], f32)
        nc.sync.dma_start(out=sb_W2, in_=dW2[:, :])
        sb_L = consts.tile([P, NT, D], f32)
        nc.sync.dma_start(out=sb_L, in_=dL[:, :].rearrange("(t p) d -> p t d", p=P))
        sb_L0 = consts.tile([P, NT, D], f32)
        nc.sync.dma_start(out=sb_L0, in_=dL0[:, :].rearrange("(t p) d -> p t d", p=P))

        sb_B1b = sb_B2 = None
        if has_b1:
            sb_B1b = consts.tile([1, D], bf16)
            nc.sync.dma_start(out=sb_B1b, in_=dB1[:, :])
        if has_b2:
            sb_B2 = consts.tile([1, D], f32)
            nc.sync.dma_start(out=sb_B2, in_=dB2[:, :])
        gam_bc = bet_bc = None
        if has_gamma:
            gam_bc = consts.tile([P, D], f32)
            src = dGAM[:, :]
            nc.sync.dma_start(
                out=gam_bc,
                in_=bass.AP(tensor=src.tensor, offset=src.offset,
                            ap=[[0, P], src.ap[1]]),
            )
        if has_beta:
            bet_bc = consts.tile([P, D], f32)
            src = dBET[:, :]
            nc.sync.dma_start(
                out=bet_bc,
                in_=bass.AP(tensor=src.tensor, offset=src.offset,
                            ap=[[0, P], src.ap[1]]),
            )

        ones_f = consts.tile([P, 1], f32)
        nc.vector.memset(ones_f, 1.0)
        ones_b = consts.tile([P, 1], bf16)
        nc.vector.memset(ones_b, 1.0)
        sb_eps = consts.tile([P, 1], f32)
        nc.vector.memset(sb_eps, LN_EPS)
        ones_row = consts.tile([1, D], bf16)
        nc.vector.memset(ones_row, 1.0)
        c2048 = consts.tile([1, 1], f32)
        nc.vector.memset(c2048, float(M))

        sb_E1 = consts.tile([P, NT, M], bf16)
        sb_E2 = consts.tile([P, NT, M], bf16)
        sb_S1Tb = consts.tile([P, M], bf16)
        sb_S2Tb = consts.tile([P, M], bf16)

        # ---- Phase 1: A^T tiles -> exp (E1 + den1) -> F' -> S1T; transpose ----
        ps_s1t = psS.tile([P, M], f32, tag="psS")
        for t in range(NT):
            tr = bass.ts(t, P)
            dparts = scal.tile([P, M // CH], f32, tag="dparts")
            for c in range(M // CH):
                pa = psA.tile([P, CH], f32, tag="psA")
                for h in range(CH // MMN):
                    ci = c * (CH // MMN) + h
                    nc.tensor.matmul(
                        pa[:, bass.ts(h, MMN)], lhsT=sb_FTb[:, tr],
                        rhs=sb_HTc[ci], start=True, stop=True,
                    )
                nc.scalar.activation(
                    out=sb_E1[:, t, bass.ts(c, CH)], in_=pa,
                    func=mybir.ActivationFunctionType.Exp,
                    bias=sb_NEG, scale=1.0,
                    accum_out=dparts[:, c : c + 1],
                )
            # transpose E1 tile -> E2 column block (idle DMA engines)
            nc.sync.dma_start_transpose(out=sb_E2[:, :, tr], in_=sb_E1[:, t, :])
            den = scal.tile([P, 1], f32, tag="den")
            nc.vector.reduce_sum(out=den, in_=dparts, axis=mybir.AxisListType.X)
            rec = scal.tile([P, 1], f32, tag="rec")
            nc.vector.reciprocal(out=rec, in_=den)
            fp = fpp.tile([P, D], bf16, tag="fp")
            nc.vector.tensor_scalar_mul(out=fp, in0=sb_F[:, t, :], scalar1=rec)
            for h in range(M // MMN):
                hs = bass.ts(h, MMN)
                nc.tensor.matmul(
                    ps_s1t[:, hs], lhsT=fp, rhs=sb_E1[:, t, hs],
                    start=(t == 0), stop=(t == NT - 1),
                )
        s1t_evac = nc.vector.tensor_copy(out=sb_S1Tb, in_=ps_s1t)

        # ---- den2[i] = column sums of E1 via ones-matmuls; each group's
        # [1, 512] PSUM row is scattered to per-partition layout by a DMA ----
        # Explicitly ordered after phase 1 so the accumulation groups don't
        # grab a psA slot mid-phase and starve the A-chunk pipeline.
        den2p = consts.tile([P, NT], f32)
        den2rec = consts.tile([P, NT], f32)
        sb_d2row = consts.tile([1, M], f32)
        ngrp = MMN // P
        for c in range(M // MMN):
            ps_den = psA.tile([1, MMN], f32, tag="psA")
            for t in range(NT):
                mm = nc.tensor.matmul(
                    ps_den, lhsT=ones_b, rhs=sb_E1[:, t, bass.ts(c, MMN)],
                    start=(t == 0), stop=(t == NT - 1),
                )
                if t == 0:
                    add_dep_helper(mm.ins, s1t_evac.ins, sync=False,
                                   reason="den2 group after phase 1")
            row = sb_d2row[:, bass.ts(c, MMN)]
            nc.vector.tensor_copy(out=row, in_=ps_den)
            for tt in range(ngrp):
                tcol = c * ngrp + tt
                nc.gpsimd.dma_start(
                    out=den2p[:, tcol : tcol + 1],
                    in_=sb_d2row[:, bass.ds(tcol * P, P)],
                )
            nc.vector.reciprocal(out=den2rec[:, bass.ts(c, ngrp)],
                                 in_=den2p[:, bass.ts(c, ngrp)])

        # ---- Phase 2: H' groups @ E2 -> S2T accumulation ----
        ps_s2t = psS.tile([P, M], f32, tag="psS")
        for c in range(NT):
            hp = fpp.tile([P, D], bf16, tag="fp")
            nc.vector.tensor_scalar_mul(out=hp, in0=sb_Hp[:, c, :],
                                        scalar1=den2rec[:, c : c + 1])
            for h in range(M // MMN):
                hs = bass.ts(h, MMN)
                nc.tensor.matmul(
                    ps_s2t[:, hs], lhsT=hp, rhs=sb_E2[:, c, hs],
                    start=(c == 0), stop=(c == NT - 1),
                )
        nc.vector.tensor_copy(out=sb_S2Tb, in_=ps_s2t)

        # ---- MLP phases: pre-act -> LayerNorm -> ReLU -> column-sum ----
        def mlp_colsum(xslice, sTb, pre_all, mvall, racc, r_sb):
            for t in range(NT):
                tr = bass.ts(t, P)
                pre = psA.tile([P, D], f32, tag="psA")
                nc.tensor.matmul(pre, lhsT=xslice(t), rhs=sb_W1b[:, 0, :],
                                 start=True, stop=False)
                nc.tensor.matmul(pre, lhsT=sTb[:, tr], rhs=sb_W1b[:, 1, :],
                                 start=False, stop=not has_b1)
                if has_b1:
                    nc.tensor.matmul(pre, lhsT=ones_row, rhs=sb_B1b,
                                     start=False, stop=True)
                nc.scalar.copy(out=pre_all[:, t, :], in_=pre)
                stats = scal.tile([P, 6], f32, tag="stats")
                nc.vector.bn_stats(out=stats, in_=pre_all[:, t, :])
                nc.vector.bn_aggr(out=mvall[:, t, :], in_=stats)
            lnv = scal.tile([P, NT], f32, tag="lnv")
            nc.scalar.activation(out=lnv, in_=mvall[:, :, 1],
                                 func=mybir.ActivationFunctionType.Ln,
                                 bias=sb_eps, scale=1.0)
            rstd_all = consts.tile([P, NT], f32)
            nc.scalar.activation(out=rstd_all, in_=lnv,
                                 func=mybir.ActivationFunctionType.Exp,
                                 scale=-0.5)
            for t in range(NT):
                tt = mlpt.tile([P, D], f32, tag="tt")
                nc.vector.tensor_scalar(
                    out=tt, in0=pre_all[:, t, :], scalar1=mvall[:, t, 0:1],
                    scalar2=rstd_all[:, t : t + 1],
                    op0=mybir.AluOpType.subtract, op1=mybir.AluOpType.mult,
                )
                if has_gamma:
                    nc.vector.tensor_mul(out=tt, in0=tt, in1=gam_bc)
                if has_beta:
                    nc.vector.tensor_add(out=tt, in0=tt, in1=bet_bc)
                if t == 0:
                    nc.vector.tensor_scalar_max(out=racc, in0=tt, scalar1=0.0)
                else:
                    nc.vector.scalar_tensor_tensor(
                        out=racc, in0=tt, scalar=0.0, in1=racc,
                        op0=mybir.AluOpType.max, op1=mybir.AluOpType.add,
                    )
            ps_r = psA.tile([P, 1], f32, tag="psA")
            nc.tensor.matmul(ps_r, lhsT=racc, rhs=ones_f, start=True, stop=True)
            nc.vector.tensor_copy(out=r_sb, in_=ps_r)

        nchunk = MMN // P

        pre_all2 = consts.tile([P, NT, D], bf16)
        mvall2 = consts.tile([P, NT, 2], f32)
        racc2 = consts.tile([P, D], f32)
        r2_sb = consts.tile([P, 1], f32)
        mlp_colsum(lambda t: sb_HTc[t // nchunk][:, bass.ts(t % nchunk, P)],
                   sb_S1Tb, pre_all2, mvall2, racc2, r2_sb)

        pre_all1 = consts.tile([P, NT, D], bf16)
        mvall1 = consts.tile([P, NT, 2], f32)
        racc1 = consts.tile([P, D], f32)
        r1_sb = consts.tile([P, 1], f32)
        mlp_colsum(lambda t: sb_FTb[:, bass.ts(t, P)],
                   sb_S2Tb, pre_all1, mvall1, racc1, r1_sb)

        # ---- pooled vectors ----
        pcat = consts.tile([P, 2], f32)

        def lcolsum(sb_src, l_sb):
            ps_l = psA.tile([P, 1], f32, tag="psA")
            for t in range(NT):
                nc.tensor.matmul(ps_l, lhsT=sb_src[:, t, :], rhs=ones_f,
                                 start=(t == 0), stop=(t == NT - 1))
            nc.vector.tensor_copy(out=l_sb, in_=ps_l)

        l2_sb = consts.tile([P, 1], f32)
        lcolsum(sb_L, l2_sb)
        l1_sb = consts.tile([P, 1], f32)
        lcolsum(sb_L0, l1_sb)

        def pvec(r_sb, l_sb, out_slice):
            ps_p = psA.tile([P, 1], f32, tag="psA")
            nc.tensor.matmul(ps_p, lhsT=sb_W2, rhs=r_sb, start=True,
                             stop=not has_b2)
            if has_b2:
                nc.tensor.matmul(ps_p, lhsT=sb_B2, rhs=c2048, start=False,
                                 stop=True)
            nc.vector.tensor_add(out=out_slice, in0=ps_p, in1=l_sb)

        pvec(r2_sb, l2_sb, pcat[:, 1:2])
        pvec(r1_sb, l1_sb, pcat[:, 0:1])

        # ---- final: out = s12 / sqrt(s11*s22) via exp(-0.5*ln(.)) ----
        ps_d1 = psA.tile([1, 2], f32, tag="psA")
        nc.tensor.matmul(ps_d1, lhsT=pcat[:, 0:1], rhs=pcat, start=True, stop=True)
        ps_d2 = psA.tile([1, 1], f32, tag="psA")
        nc.tensor.matmul(ps_d2, lhsT=pcat[:, 1:2], rhs=pcat[:, 1:2],
                         start=True, stop=True)
        dots = consts.tile([1, 4], f32)
        nc.vector.tensor_copy(out=dots[:, 0:2], in_=ps_d1)   # s11, s12
        nc.vector.tensor_copy(out=dots[:, 2:3], in_=ps_d2)   # s22
        q = consts.tile([1, 1], f32)
        nc.vector.tensor_mul(out=q, in0=dots[:, 0:1], in1=dots[:, 2:3])
        nc.vector.tensor_scalar_max(out=q, in0=q, scalar1=1e-30)
        lq = consts.tile([1, 1], f32)
        nc.scalar.activation(out=lq, in_=q,
                             func=mybir.ActivationFunctionType.Ln)
        rq = consts.tile([1, 1], f32)
        nc.scalar.activation(out=rq, in_=lq,
                             func=mybir.ActivationFunctionType.Exp,
                             scale=-0.5)
        res = consts.tile([1, 1], f32)
        nc.vector.tensor_mul(out=res, in0=dots[:, 1:2], in1=rq)
        nc.sync.dma_start(out=dOUT[:, :], in_=res)

    split_waits(nc)
    return nc


_BUILD_CACHE = {}


def _get_nc(flags):
    if flags not in _BUILD_CACHE:
        _BUILD_CACHE[flags] = build_nc(*flags)
    return _BUILD_CACHE[flags]


def kernel(x, edge_attr, W1, b1, gamma, beta, W2, b2, gid, edge_index, batch):
    import ml_dtypes

    nbf16 = ml_dtypes.bfloat16
    x = np.asarray(x, dtype=np.float32)
    W1 = np.asarray(W1, dtype=np.float32)
    b1 = np.asarray(b1, dtype=np.float32)
    gamma = np.asarray(gamma, dtype=np.float32)
    beta = np.asarray(beta, dtype=np.float32)
    W2 = np.asarray(W2, dtype=np.float32)
    b2 = np.asarray(b2, dtype=np.float32)
    gid = int(np.asarray(gid))
    ei0 = np.asarray(edge_index)[0]
    b = np.asarray(batch)

    N, Dx = x.shape
    assert Dx == D
    deg = np.bincount(ei0, minlength=N)
    mask = deg > 1
    G = int(b.max()) + 1
    assert G == 8
    hd_idx = np.where(mask)[0]
    fhb = b[hd_idx]
    Mtot = hd_idx.size
    assert Mtot % G == 0 and np.array_equal(
        fhb, np.repeat(np.arange(G), Mtot // G)
    )
    assert Mtot // G == M

    gxf_idx = np.where(mask & (b == gid))[0]
    assert gxf_idx.size == M
    F = np.ascontiguousarray(x[gxf_idx])
    FTb = np.ascontiguousarray(F.T).astype(nbf16)
    lo0_idx = np.where((~mask) & (b == gid))[0]
    assert lo0_idx.size == M
    L0 = np.ascontiguousarray(x[lo0_idx])

    # per-core scalar exp shift: gid core centers the window on the row-norm
    # range (diagonal dominates there); others use a constant
    sq = (F.astype(np.float64) ** 2).sum(1)
    c_gid = float((sq.max() + sq.min()) / 2.0)

    flags = (
        bool(np.any(b1 != 0.0)),
        bool(np.any(b2 != 0.0)),
        bool(np.any(gamma != 1.0)),
        bool(np.any(beta != 0.0)),
    )
    has_b1, has_b2, has_gamma, has_beta = flags
    nc = _get_nc(flags)

    W1b = W1.astype(nbf16)
    in_maps = []
    for g in range(G):
        sel_h = mask & (b == g)
        sel_l = (~mask) & (b == g)
        assert sel_h.sum() == M and sel_l.sum() == M
        H = np.ascontiguousarray(x[sel_h])
        L = np.ascontiguousarray(x[sel_l])
        cshift = c_gid if g == gid else SHIFT0
        im = {
            "HTb": np.ascontiguousarray(H.T).astype(nbf16),
            "FTb": FTb,
            "H": H,
            "F": F,
            "L": L,
            "L0": L0,
            "W1b": W1b,
            "W2": W2,
            "NEG": np.full((P, 1), -cshift, np.float32),
        }
        if has_b1:
            im["B1b"] = b1.reshape(1, D).astype(nbf16)
        if has_b2:
            im["B2"] = b2.reshape(1, D).astype(np.float32)
        if has_gamma:
            im["GAM"] = gamma.reshape(1, D).astype(np.float32)
        if has_beta:
            im["BET"] = beta.reshape(1, D).astype(np.float32)
        in_maps.append(im)

    trace_dir = os.environ.get("ADAGMN_TRACE", "")
    if trace_dir:
        res = run_bass_kernel_spmd(
            nc, in_maps, core_ids=list(range(G)), trace=True, tmpdir=trace_dir
        )
        print(f"HW exec time: {res.exec_time_ns} ns")
    else:
        res = run_bass_kernel_spmd(nc, in_maps, core_ids=list(range(G)))
    out = np.array([res.results[g]["out"][0, 0] for g in range(G)], np.float32)
    return out
